# revision 19
# baseline (speedup 1.0000x reference)
"""LMMD (DSAN local MMD) loss on 8 Trainium2 NeuronCores — triangle + fp8 V3.

Math (reference):
    X = concat(source, target)                    # [N=4096, D=1024]
    l2[i,j] = max(|x_i|^2 + |x_j|^2 - 2 x_i.x_j, 0)
    bw      = sum(l2) / (N^2 - N) / 4
    K       = sum_q exp(-l2 / (bw * 2^q)),  q = 0..4
    loss    = sum_c v_c^T K v_c / 12,  V = [s_norm; -t_norm]  (rank-12 weights)

V3 design (vs V2 triangle/fp8 baseline):
  * Bias folding: the j-side factor exp(-c_q sq_j) moves from the ACT exp
    bias into per-q host-built vt tables (vt2_q = 2 V f_q).  The es levels
    become pure powers g^(2^k) of g = exp(2 c4 G), so the group's four
    slots share ONE wide ACT exp (no per-slot bias) — one contiguous PSUM
    gram tile per group feeds it.
  * Square rebalance across engines (cost-model rates: DVE 0.52, ACT 0.83,
    Pool 1.98 ns/col): DVE squares e3/e2 + leftovers, ACT additionally
    computes e1[:, :aw] directly from the gram as exp(8s G), Pool squares
    e0[:, -pw:].  Last group skips Pool (its latency would extend the tail).
  * Scales ride in btab (bf16) — no separate ftab DMA.
  * rq is ONE PSUM tile [128, 5*4*C]; output DMA goes directly PSUM->DRAM
    (no staging copies).
"""

import numpy as np
import ml_dtypes

import concourse.bass as bass
from concourse import bacc
import concourse.mybir as mybir
import concourse.tile as tile
from concourse.bass_utils import run_bass_kernel_spmd

B = 2048
D = 1024
C = 12
NCORES = 8
N = 2 * B                 # 4096 total samples
IPC = N // NCORES         # 512 own columns (i) per core
NT = N // 128             # 32 j-tiles
NKC = D // 128            # 8 contraction chunks
NKP = NKC // 2            # 4 DoubleRow chunk-pairs
NQ = 5                    # kernels in the RBF mixture
NGROUPS = 8               # slot groups: own + 7 foreign cores
OWN_G = 0                 # own group first (cheapest start: no weight DMA)
WLAG = 3                  # groups of lag between es production and weighted use

# (block_off, block_end) in 128-col i-blocks, per slot-in-group
OWN_SPANS = [(0, 4), (1, 4), (2, 4), (3, 4)]
FOREIGN_PAT = [(0, 2), (0, 2), (2, 4), (2, 4)]

# per-group elementwise split (cols): a1 = ACT's exp(8sG) share of e1,
# a0 = ACT's exp(16sG) share of e0 (rest of e0 squared by Pool from DVE's
# e1 region — a0 >= a1 keeps Pool off the ACT-written e1 cols).
A1_FOREIGN, A0_FOREIGN = 200, 200
A1_OWN, A0_OWN = 256, 256

# btab layout (bf16): vt2 [5*NT*C] | vt1 [5*4*C] | scales [4]
VT2_COLS = NQ * NT * C
VT1_COLS = NQ * 4 * C
BT_COLS = VT2_COLS + VT1_COLS + 4

F8NP = ml_dtypes.float8_e4m3
BFNP = ml_dtypes.bfloat16

_BUILT = None             # program is input-independent


def _slot_geom(g, sl):
    if g == OWN_G:
        off, end = OWN_SPANS[sl]
    else:
        off, end = FOREIGN_PAT[sl]
    return off, end


OWN_EBASE = [0, 640, 1024, 512]   # keeps every slot slice inside one PSUM bank


def _ebase(g, sl):
    # column base of slot sl inside the group's batched gram/e tiles.
    # A matmul output must not cross a 512-fp32 PSUM bank boundary, so the
    # own group's 512/384/256/128 spans are packed [0:512|640:1024|1024:1280|
    # 512:640] instead of cumulatively.
    if g == OWN_G:
        return OWN_EBASE[sl]
    bases = [0]
    for s in range(1, 4):
        o, e = _slot_geom(g, s - 1)
        bases.append(bases[-1] + (e - o) * 128)
    return bases[sl]


def _gsplit(g):
    gw = 1280 if g == OWN_G else 1024
    a1, a0 = (A1_OWN, A0_OWN) if g == OWN_G else (A1_FOREIGN, A0_FOREIGN)
    return gw, a1, a0


def _build_program():
    fp32 = mybir.dt.float32
    bf16 = mybir.dt.bfloat16
    f8 = mybir.dt.float8e4
    Exp = mybir.ActivationFunctionType.Exp
    DR = mybir.MatmulPerfMode.DoubleRow

    nc = bacc.Bacc()
    # host-pretransposed: xtb[p, k, s*128+j] = X[jseq[s]*128+j, k*128+p]
    xtb = nc.declare_dram_parameter("xtb", [128, NKC, NT * 128], f8, isOutput=False)
    btab = nc.declare_dram_parameter("btab", [128, BT_COLS], bf16, isOutput=False)
    # flipped weighted layout: rows = i within own 128-block, cols = (q, block, cls)
    rout = nc.declare_dram_parameter("r_out", [128, NQ * 4 * C], fp32, isOutput=True)

    with tile.TileContext(nc) as tc:
        with (
            tc.tile_pool(name="singles", bufs=1) as singles,
            tc.tile_pool(name="wpool", bufs=3) as wpool,
            tc.tile_pool(name="epool", bufs=5) as epool,
            tc.tile_pool(name="gpsum", bufs=2, space="PSUM") as gpsum,
            tc.tile_pool(name="rqpsum", bufs=1, space="PSUM") as rqpsum,
        ):
            # own i-columns = slots 0..3 of xtb, first on the serialized DMA
            # path (gram slot 0 starts after the first half).
            own_sb = singles.tile([128, NKC, IPC], f8)
            nc.sync.dma_start(out=own_sb[:, 0 : NKC // 2, :], in_=xtb[:, 0 : NKC // 2, 0:IPC])
            nc.sync.dma_start(out=own_sb[:, NKC // 2 : NKC, :], in_=xtb[:, NKC // 2 : NKC, 0:IPC])
            btab_sb = singles.tile([128, BT_COLS], bf16)
            # fp32 scale/bias staging: [scl_e4, scl_e1, scl_e0, zero].  The
            # scales ride in btab's tail; fetch just those 4 cols early (the
            # bulk vt DMA would otherwise delay wg1 / the first exp).
            nc.sync.dma_start(
                out=btab_sb[:, VT2_COLS + VT1_COLS :],
                in_=btab[:, VT2_COLS + VT1_COLS :],
            )
            scl_s = singles.tile([128, 4], fp32)
            nc.vector.tensor_copy(scl_s, btab_sb[:, VT2_COLS + VT1_COLS :])
            warm = singles.tile([128, 4], fp32)
            # Dummy ACT op: loads the Exp table early and absorbs the DVE
            # wait so loop Exp ops only ever need the PE wait.
            nc.scalar.activation(warm, scl_s, Exp)

            # rq[p, q*48 + b*C + cls] accumulates R_q over j for own block b.
            # Two tiles (q>=2 | q<=1) so the high-q drain copies overlap the
            # remaining low-q weighted matmuls; each tile = one PSUM bank.
            rq_hi = rqpsum.tile([128, 3 * 4 * C], fp32, tag="rqh", name="rq_hi")
            rq_lo = rqpsum.tile([128, 2 * 4 * C], fp32, tag="rql", name="rq_lo")

            def rq_slice(q, b):
                if q >= 2:
                    return rq_hi[:, (q - 2) * 4 * C + b * C : (q - 2) * 4 * C + (b + 1) * C]
                return rq_lo[:, q * 4 * C + b * C : q * 4 * C + (b + 1) * C]

            def emit_weighted(g, es):
                # Flipped orientation: es block stationary, vt moving.
                # q-major, q=4 first so PE chases the squaring chain.
                for q in range(NQ - 1, -1, -1):
                    for sl in range(4):
                        slot = 4 * g + sl
                        off, end = _slot_geom(g, sl)
                        eb = _ebase(g, sl)
                        for b in range(off, end):
                            if g == OWN_G and b == sl:
                                base = VT2_COLS + (q * 4 + sl) * C
                            else:
                                base = (q * NT + slot) * C
                            vtb = btab_sb[:, base : base + C]
                            col = eb + (b - off) * 128
                            # PSUM has_written: start clears the WHOLE bank's
                            # bits, so only the first matmul into each rq bank
                            # may set it — every slice then first-touch-
                            # overwrites (bit clear) and accumulates after.
                            nc.tensor.matmul(
                                rq_slice(q, b),
                                lhsT=es[q][:, col : col + 128],
                                rhs=vtb,
                                start=(g == 0 and q in (NQ - 1, 1) and sl == 0 and b == 0),
                                stop=(g == NGROUPS - 1 and q in (2, 0) and sl == 3 and b == 3),
                            )

            pending = []
            for g in range(NGROUPS):
                gw, a1, a0 = _gsplit(g)
                if g == OWN_G:
                    wsrc = own_sb
                else:
                    wg = wpool.tile([128, NKC, 512], f8, tag="wg", name=f"w{g}")
                    src0 = g * 512
                    nc.sync.dma_start(out=wg, in_=xtb[:, :, src0 : src0 + 512])
                    if g == 1:
                        # vt tables land after wg1; first consumer is
                        # weighted(0) at ~8us so wg1 wins the DMA path.
                        # Weighted matmuls read btab_sb directly (one DMA
                        # sem, many waiters — no staging copy needed).
                        nc.sync.dma_start(
                            out=btab_sb[:, 0 : VT2_COLS + VT1_COLS],
                            in_=btab[:, 0 : VT2_COLS + VT1_COLS],
                        )
                    wsrc = wg
                # one contiguous PSUM gram tile for the whole group
                gt = gpsum.tile([128, 1280], fp32, tag="g", name=f"g{g}")
                for sl in (range(3, -1, -1) if g == OWN_G else range(4)):
                    off, end = _slot_geom(g, sl)
                    span = (end - off) * 128
                    eb = _ebase(g, sl)
                    for m in range(NKP):
                        nc.tensor.matmul(
                            gt[:, eb : eb + span],
                            lhsT=wsrc[:, 2 * m : 2 * m + 2, sl * 128 : (sl + 1) * 128],
                            rhs=own_sb[:, 2 * m : 2 * m + 2, off * 128 : end * 128],
                            start=(m == 0),
                            stop=(m == NKP - 1),
                            perf_mode=DR,
                        )
                es = {q: epool.tile([128, 1280], bf16, tag=f"e{q}", name=f"e{q}g{g}") for q in range(NQ)}
                # Decoupled engine chains (no mid-chain cross-engine joins):
                # ACT: three independent exps straight from the gram; DVE:
                # e3->e2->e1 squaring; Pool: e0 tail squared from DVE's e1.
                nc.scalar.activation(
                    es[4][:, 0:gw], gt[:, 0:gw], Exp,
                    bias=scl_s[:, 3:4], scale=scl_s[:, 0:1],
                )
                nc.scalar.activation(
                    es[1][:, 0:a1], gt[:, 0:a1], Exp,
                    bias=scl_s[:, 3:4], scale=scl_s[:, 1:2],
                )
                nc.scalar.activation(
                    es[0][:, 0:a0], gt[:, 0:a0], Exp,
                    bias=scl_s[:, 3:4], scale=scl_s[:, 2:3],
                )
                nc.vector.tensor_mul(es[3][:, 0:gw], es[4][:, 0:gw], es[4][:, 0:gw])
                nc.vector.tensor_mul(es[2][:, 0:gw], es[3][:, 0:gw], es[3][:, 0:gw])
                nc.vector.tensor_mul(es[1][:, a1:gw], es[2][:, a1:gw], es[2][:, a1:gw])
                if g == NGROUPS - 1:
                    # keep slow Pool off the tail-latency path
                    nc.vector.tensor_mul(es[0][:, a0:gw], es[1][:, a0:gw], es[1][:, a0:gw])
                else:
                    nc.gpsimd.tensor_mul(es[0][:, a0:gw], es[1][:, a0:gw], es[1][:, a0:gw])
                pending.append((g, es))
                if len(pending) > WLAG:
                    emit_weighted(*pending.pop(0))
            for item in pending:
                emit_weighted(*item)

            # tail: the q>=2 tile drains (ACT) while q1/q0 matmuls still run,
            # then one DVE copy for the low tile and a single DMA.
            stg = singles.tile([128, NQ * 4 * C], fp32)
            Copy = mybir.ActivationFunctionType.Copy
            nc.scalar.activation(stg[:, 2 * 4 * C :], rq_hi, Copy)
            nc.vector.tensor_copy(stg[:, 0 : 2 * 4 * C], rq_lo)
            nc.sync.dma_start(out=rout[:], in_=stg)

    nc.compile()
    return nc


def _jseq(c):
    seq = list(range(4 * c, 4 * c + 4))
    for d in range(NCORES):
        if d == c:
            continue
        if d > c:
            seq += [4 * d, 4 * d + 1, 4 * d + 2, 4 * d + 3]
        else:
            seq += [4 * d + 2, 4 * d + 3, 4 * d, 4 * d + 1]
    return seq


def _prep(source, target, source_label, target_logits):
    X = np.concatenate([np.asarray(source), np.asarray(target)], axis=0)
    X64 = X.astype(np.float64)
    sq = np.einsum("nd,nd->n", X64, X64)
    colsum = X64.sum(axis=0)
    sum_l2 = 2.0 * N * sq.sum() - 2.0 * (colsum @ colsum)
    bw = sum_l2 / (N * N - N) / (2.0 ** (NQ // 2))
    cq = np.array([1.0 / (bw * 2.0**q) for q in range(NQ)])  # [5]

    sl = np.asarray(source_label, np.float64)
    tl = np.asarray(target_logits, np.float64)
    ssum = sl.sum(0)
    s_norm = np.where(ssum > 0, sl / np.where(ssum > 0, ssum, 1.0), 0.0)
    tsum = tl.sum(0)
    t_norm = np.where(tsum > 0, tl / np.where(tsum > 0, tsum, 1.0), 0.0)
    s_pres = np.zeros(C)
    np.add.at(s_pres, sl.argmax(1), 1.0)
    t_pres = np.zeros(C)
    np.add.at(t_pres, tl.argmax(1), 1.0)
    common = ((s_pres > 0) & (t_pres > 0)).astype(np.float64)
    V = np.concatenate([s_norm * common, -t_norm * common], axis=0)  # [N, C]

    # j-side RBF bias folded into the vt tables: vt2_q = 2 V f_q
    fq = np.exp(-np.outer(cq, sq))                        # [5, N]

    # fp8 X^T in [p, k, jcol] layout (global j order; per-core slot perm later)
    X8 = X.astype(F8NP)                                   # [N, D]
    xt8 = np.ascontiguousarray(
        X8.T.reshape(NKC, 128, N).transpose(1, 0, 2)      # [128, 8, N]
    )
    return X, sq, cq, V, fq, xt8


def _core_inputs(c, cq, V, fq, xt8):
    seq = _jseq(c)
    # xtb: permute j-tiles into slot order
    xtb = np.ascontiguousarray(
        xt8.reshape(128, NKC, NT, 128)[:, :, seq, :].reshape(128, NKC, NT * 128)
    )
    # Vq[q, j, cls] = V[j, cls] * f_q[j], j in slot order
    Vt = V.reshape(NT, 128, C)[seq]                       # [NT, 128, C]
    fqt = fq.reshape(NQ, NT, 128)[:, seq]                 # [NQ, NT, 128]
    Vq = Vt[None] * fqt[..., None]                        # [NQ, NT, 128, C]
    vt2 = (2.0 * Vq).transpose(2, 0, 1, 3).reshape(128, NQ * NT * C)
    vt1 = Vq[:, :4].transpose(2, 0, 1, 3).reshape(128, NQ * 4 * C)
    scl = np.zeros((128, 4))
    scl[:, 0] = 2.0 * cq[4]
    scl[:, 1] = 2.0 * cq[1]
    scl[:, 2] = 2.0 * cq[0]
    btab = np.ascontiguousarray(
        np.concatenate([vt2, vt1, scl], axis=1)
    ).astype(BFNP)
    return {"xtb": xtb, "btab": btab}


def _postprocess(results, sq, cq, V):
    # loss = 1/12 sum_q sum_i alpha_q[i] * (sum_cls V[i,cls] R_q[cls,i])
    loss = 0.0
    for c in range(NCORES):
        # r[p, q, b, cls] = R_q[cls, i] at i = 512c + 128b + p
        r = np.asarray(results[c]["r_out"], np.float64).reshape(128, NQ, 4, C)
        gi = c * IPC + np.arange(IPC)
        Vc = V[gi].reshape(4, 128, C)                     # [b, p, cls]
        alpha = np.exp(-np.outer(cq, sq[gi])).reshape(NQ, 4, 128)
        loss += np.einsum("qbp,bpc,pqbc->", alpha, Vc, r)
    return loss / C


def _run(in_maps, trace=False, **kw):
    global _BUILT
    if _BUILT is None:
        _BUILT = _build_program()
    return run_bass_kernel_spmd(_BUILT, in_maps, list(range(NCORES)), trace=trace, **kw)


def kernel(source, target, source_label, target_logits, _trace=False, _ret_bkr=False):
    X, sq, cq, V, fq, xt8 = _prep(source, target, source_label, target_logits)
    in_maps = [_core_inputs(c, cq, V, fq, xt8) for c in range(NCORES)]
    bkr = None
    for attempt in range(3):
        try:
            bkr = _run(in_maps, trace=_trace)
            break
        except Exception:
            # transient device wedge (NRT_EXEC_UNIT_UNRECOVERABLE) — back off
            # briefly and retry; the device recovers on a fresh session
            if attempt == 2:
                raise
            import time as _time

            _time.sleep(2.0)
    loss = _postprocess(bkr.results, sq, cq, V)
    out = np.float32(loss)
    if _ret_bkr:
        return out, bkr
    return out


# revision 20
# speedup vs baseline: 1.0504x; 1.0504x over previous
"""LMMD (DSAN local MMD) loss on 8 Trainium2 NeuronCores — cyclic-support V5.

Math (reference):
    X = concat(source, target)                    # [N=4096, D=1024]
    l2[i,j] = max(|x_i|^2 + |x_j|^2 - 2 x_i.x_j, 0)
    bw      = sum(l2) / (N^2 - N) / 4
    K       = sum_q exp(-l2 / (bw * 2^q)),  q = 0..4
    loss    = sum_c v_c^T K v_c / 12,  V = [s_norm; -t_norm]  (rank-12 weights)

V5 design:
  * Cyclic 16-tile support: core c holds X columns for tiles
    (4c + S0) mod 32 with S0 = {0..7, 12..19}.  The 528 unordered
    128-tile pairs partition into 8 identical 68-job lists (60 weight-2
    oriented pairs covering every (difference, residue) cell once, 4
    weight-1 distance-16 jobs computed twice with opposite orientations,
    4 weight-1 diagonals), so every core runs the SAME program on a
    rotated tile set and per-core X DMA halves to 16 KB/partition.
  * Jobs stream through 9 batches (6|8x7|6 jobs).  Per batch: fp8
    DoubleRow gram into one 2-bank PSUM tile, three bias-free ACT exps
    (e4 = exp(2c4 G) full width, e1/e0 heads straight from the gram at
    8x/16x scale), DVE squaring e3/e2/e1-tail, Pool squares the e0 tail
    from DVE's e1 region only (fully decoupled engine chains).  The
    j-side RBF factor exp(-c_q sq_j) is folded into per-q bf16 vt
    tables; the i-side factor is applied on the host.
  * Weighted reduce keeps es stationary (12-wide moving vt), accumulating
    R_q[i, cls] into two PSUM tiles (q>=2 / q<=1) so the high-q drain
    overlaps the low-q matmuls.  PSUM has_written semantics: one
    start per bank, first-touch overwrites via cleared bits.
"""

import numpy as np
import ml_dtypes

import concourse.bass as bass
from concourse import bacc
import concourse.mybir as mybir
import concourse.tile as tile
from concourse.bass_utils import run_bass_kernel_spmd

B = 2048
D = 1024
C = 12
NCORES = 8
N = 2 * B                 # 4096 total samples
NT = N // 128             # 32 j-tiles
NKC = D // 128            # 8 contraction chunks
NKP = NKC // 2            # 4 DoubleRow chunk-pairs
NQ = 5
M = 16                    # tiles in the cyclic support
NI = 8                    # i-side slots (positions 0..7)
WLAG = 3                  # batches of lag between es production and weighted

S0 = list(range(0, 8)) + list(range(12, 20))

# btab layout (bf16): vt2 [5*M*C] | vt1 [5*8*C] | scales [4]
VT2_COLS = NQ * M * C
VT1_COLS = NQ * 8 * C
BT_COLS = VT2_COLS + VT1_COLS + 4

F8NP = ml_dtypes.float8_e4m3
BFNP = ml_dtypes.bfloat16

_BUILT = None


def _plan_jobs():
    """Deterministic job plan: 68 (jpos, ipos, weight, vt1slot) tuples in
    S0-local positions, every global pair covered exactly once."""
    import itertools

    Sset = set(S0)
    pos = {t: i for i, t in enumerate(S0)}
    ILOCAL = set(range(0, 8))
    pairs = []
    for a, b in itertools.combinations(S0, 2):
        if a not in ILOCAL and b not in ILOCAL:
            continue
        d = (b - a) % 32
        cells = set()
        for (base, dd) in ((a, d), (b, (32 - d) % 32)):
            if 1 <= dd <= 15:
                cells.add((dd, base % 4))
        if cells:
            pairs.append(((a, b), sorted(cells)))
    cells_needed = [(d, r) for d in range(1, 16) for r in range(4)]
    cell_idx = {c: i for i, c in enumerate(cells_needed)}
    adj = [[] for _ in cells_needed]
    for pi, (fs, cells) in enumerate(pairs):
        for cc in cells:
            if cc in cell_idx:
                adj[cell_idx[cc]].append(pi)
    for ci in range(len(adj)):
        adj[ci].sort(key=lambda pi: max(pairs[pi][0]))
    match_pair = {}
    match_cell = [None] * len(cells_needed)

    def aug(ci, seen):
        for pi in adj[ci]:
            if pi in seen:
                continue
            seen.add(pi)
            if pi not in match_pair or aug(match_pair[pi], seen):
                match_pair[pi] = ci
                match_cell[ci] = pi
                return True
        return False

    for ci in range(len(cells_needed)):
        assert aug(ci, set())
    jobs = []
    for ci, pi in enumerate(match_cell):
        (a, b) = pairs[pi][0]
        i_t = a if a in ILOCAL else b
        j_t = b if i_t == a else a
        jobs.append((pos[j_t], pos[i_t], 2, -1))
    for x in range(4):                       # d16, weight 1, computed twice
        jobs.append((pos[x + 16], pos[x], 1, x))
    for x in range(4):                       # diagonal, weight 1
        jobs.append((pos[x], pos[x], 1, 4 + x))
    # order by data arrival (4-position DMA chunks), then j for locality
    jobs.sort(key=lambda jb: (max(jb[0] // 4, jb[1] // 4), jb[0], jb[1]))
    return jobs


JOBS = _plan_jobs()
BATCH_SIZES = [6, 8, 8, 8, 8, 8, 8, 8, 6]
assert sum(BATCH_SIZES) == len(JOBS) == 68
BATCHES = []
_k = 0
for bs in BATCH_SIZES:
    BATCHES.append(JOBS[_k : _k + bs])
    _k += bs
NB = len(BATCHES)


def _asplit(w):
    # ACT's exp(8sG)/exp(16sG) head widths (e1/e0); Pool squares e0[a:]
    return 128 if w <= 768 else 192


def _build_program():
    fp32 = mybir.dt.float32
    bf16 = mybir.dt.bfloat16
    f8 = mybir.dt.float8e4
    Exp = mybir.ActivationFunctionType.Exp
    Copy = mybir.ActivationFunctionType.Copy
    DR = mybir.MatmulPerfMode.DoubleRow

    nc = bacc.Bacc()
    # host-pretransposed: xtb[p, k, t*128+j] = X[(4c+S0[t])*128+j, k*128+p]
    xtb = nc.declare_dram_parameter("xtb", [128, NKC, M * 128], f8, isOutput=False)
    btab = nc.declare_dram_parameter("btab", [128, BT_COLS], bf16, isOutput=False)
    rout = nc.declare_dram_parameter("r_out", [128, NQ * NI * C], fp32, isOutput=True)

    with tile.TileContext(nc) as tc:
        with (
            tc.tile_pool(name="singles", bufs=1) as singles,
            tc.tile_pool(name="epool", bufs=5) as epool,
            tc.tile_pool(name="gpsum", bufs=3, space="PSUM") as gpsum,
            tc.tile_pool(name="rqpsum", bufs=1, space="PSUM") as rqpsum,
        ):
            xtb_sb = singles.tile([128, NKC, M * 128], f8)
            btab_sb = singles.tile([128, BT_COLS], bf16)
            # DMA stream: first batch's tiles (positions 0-3) in two k-halves
            # so gram m=0,1 starts early; scales early (tiny); remaining tile
            # chunks; the bulk vt table after the second chunk.
            nc.sync.dma_start(out=xtb_sb[:, 0:4, 0:512], in_=xtb[:, 0:4, 0:512])
            nc.sync.dma_start(out=xtb_sb[:, 4:8, 0:512], in_=xtb[:, 4:8, 0:512])
            nc.sync.dma_start(
                out=btab_sb[:, VT2_COLS + VT1_COLS :],
                in_=btab[:, VT2_COLS + VT1_COLS :],
            )
            scl_s = singles.tile([128, 4], fp32)
            nc.vector.tensor_copy(scl_s, btab_sb[:, VT2_COLS + VT1_COLS :])
            warm = singles.tile([128, 4], fp32)
            nc.scalar.activation(warm, scl_s, Exp)
            nc.sync.dma_start(out=xtb_sb[:, :, 512:1024], in_=xtb[:, :, 512:1024])
            nc.sync.dma_start(
                out=btab_sb[:, 0 : VT2_COLS + VT1_COLS],
                in_=btab[:, 0 : VT2_COLS + VT1_COLS],
            )
            nc.sync.dma_start(out=xtb_sb[:, :, 1024:1536], in_=xtb[:, :, 1024:1536])
            nc.sync.dma_start(out=xtb_sb[:, :, 1536:2048], in_=xtb[:, :, 1536:2048])

            # R accumulators: hi = q {4,3,2}, lo = q {1,0}; one bank each
            rq_hi = rqpsum.tile([128, 3 * NI * C], fp32, tag="rqh", name="rq_hi")
            rq_lo = rqpsum.tile([128, 2 * NI * C], fp32, tag="rql", name="rq_lo")

            def rq_slice(q, islot):
                if q >= 2:
                    base = ((q - 2) * NI + islot) * C
                    return rq_hi[:, base : base + C]
                base = (q * NI + islot) * C
                return rq_lo[:, base : base + C]

            first_mm = {"hi": True, "lo": True}
            n_emitted = [0]

            def emit_weighted(bi, es):
                jobs = BATCHES[bi]
                for q in range(NQ - 1, -1, -1):
                    for jj, (jpos, ipos, w, vt1slot) in enumerate(jobs):
                        if w == 2:
                            vb = (q * M + jpos) * C
                        else:
                            vb = VT2_COLS + (q * 8 + vt1slot) * C
                        key = "hi" if q >= 2 else "lo"
                        n_emitted[0] += 1
                        last = n_emitted[0] == NB and False
                        nc.tensor.matmul(
                            rq_slice(q, ipos),
                            lhsT=es[q][:, jj * 128 : (jj + 1) * 128],
                            rhs=btab_sb[:, vb : vb + C],
                            start=first_mm[key],
                            stop=(bi == NB - 1 and q in (2, 0) and jj == len(jobs) - 1),
                        )
                        first_mm[key] = False

            pending = []
            for bi, jobs in enumerate(BATCHES):
                w = len(jobs) * 128
                a = _asplit(w)
                gt = gpsum.tile([128, 1024], fp32, tag="g", name=f"g{bi}")
                for jj, (jpos, ipos, _, _) in enumerate(jobs):
                    for m in range(NKP):
                        nc.tensor.matmul(
                            gt[:, jj * 128 : (jj + 1) * 128],
                            lhsT=xtb_sb[:, 2 * m : 2 * m + 2, jpos * 128 : (jpos + 1) * 128],
                            rhs=xtb_sb[:, 2 * m : 2 * m + 2, ipos * 128 : (ipos + 1) * 128],
                            start=(m == 0),
                            stop=(m == NKP - 1),
                            perf_mode=DR,
                        )
                es = {q: epool.tile([128, 1024], bf16, tag=f"e{q}", name=f"e{q}b{bi}") for q in range(NQ)}
                nc.scalar.activation(
                    es[4][:, 0:w], gt[:, 0:w], Exp,
                    bias=scl_s[:, 3:4], scale=scl_s[:, 0:1],
                )
                nc.scalar.activation(
                    es[1][:, 0:a], gt[:, 0:a], Exp,
                    bias=scl_s[:, 3:4], scale=scl_s[:, 1:2],
                )
                nc.scalar.activation(
                    es[0][:, 0:a], gt[:, 0:a], Exp,
                    bias=scl_s[:, 3:4], scale=scl_s[:, 2:3],
                )
                nc.vector.tensor_mul(es[3][:, 0:w], es[4][:, 0:w], es[4][:, 0:w])
                nc.vector.tensor_mul(es[2][:, 0:w], es[3][:, 0:w], es[3][:, 0:w])
                nc.vector.tensor_mul(es[1][:, a:w], es[2][:, a:w], es[2][:, a:w])
                if bi == NB - 1:
                    nc.vector.tensor_mul(es[0][:, a:w], es[1][:, a:w], es[1][:, a:w])
                else:
                    nc.gpsimd.tensor_mul(es[0][:, a:w], es[1][:, a:w], es[1][:, a:w])
                pending.append((bi, es))
                if len(pending) > WLAG:
                    emit_weighted(*pending.pop(0))
            for item in pending:
                emit_weighted(*item)

            # tail: hi drains on ACT while the low-q matmuls still run, then
            # one DVE copy for lo and a single DMA.
            stg = singles.tile([128, NQ * NI * C], fp32)
            nc.scalar.activation(stg[:, 2 * NI * C :], rq_hi, Copy)
            nc.vector.tensor_copy(stg[:, 0 : 2 * NI * C], rq_lo)
            nc.sync.dma_start(out=rout[:], in_=stg)

    nc.compile()
    return nc


def _prep(source, target, source_label, target_logits):
    X = np.concatenate([np.asarray(source), np.asarray(target)], axis=0)
    X64 = X.astype(np.float64)
    sq = np.einsum("nd,nd->n", X64, X64)
    colsum = X64.sum(axis=0)
    sum_l2 = 2.0 * N * sq.sum() - 2.0 * (colsum @ colsum)
    bw = sum_l2 / (N * N - N) / (2.0 ** (NQ // 2))
    cq = np.array([1.0 / (bw * 2.0**q) for q in range(NQ)])  # [5]

    sl = np.asarray(source_label, np.float64)
    tl = np.asarray(target_logits, np.float64)
    ssum = sl.sum(0)
    s_norm = np.where(ssum > 0, sl / np.where(ssum > 0, ssum, 1.0), 0.0)
    tsum = tl.sum(0)
    t_norm = np.where(tsum > 0, tl / np.where(tsum > 0, tsum, 1.0), 0.0)
    s_pres = np.zeros(C)
    np.add.at(s_pres, sl.argmax(1), 1.0)
    t_pres = np.zeros(C)
    np.add.at(t_pres, tl.argmax(1), 1.0)
    common = ((s_pres > 0) & (t_pres > 0)).astype(np.float64)
    V = np.concatenate([s_norm * common, -t_norm * common], axis=0)  # [N, C]

    fq = np.exp(-np.outer(cq, sq))                        # [5, N]

    X8 = X.astype(F8NP)                                   # [N, D]
    xt8 = np.ascontiguousarray(
        X8.T.reshape(NKC, 128, N).transpose(1, 0, 2)      # [128, 8, N]
    )
    return X, sq, cq, V, fq, xt8


def _core_inputs(c, cq, V, fq, xt8):
    gtiles = [(4 * c + s) % 32 for s in S0]
    xtb = np.ascontiguousarray(
        xt8.reshape(128, NKC, NT, 128)[:, :, gtiles, :].reshape(128, NKC, M * 128)
    )
    # vt2[q, t] = 2 V f_q at global tile gtiles[t]
    Vt = V.reshape(NT, 128, C)[gtiles]                    # [M, 128, C]
    fqt = fq.reshape(NQ, NT, 128)[:, gtiles]              # [NQ, M, 128]
    Vq = Vt[None] * fqt[..., None]                        # [NQ, M, 128, C]
    vt2 = (2.0 * Vq).transpose(2, 0, 1, 3).reshape(128, NQ * M * C)
    # vt1 slots: 0..3 = d16 jobs (j = position 8+... tile 16+x), 4..7 = diag x
    vt1 = np.zeros((128, NQ, 8, C))
    for x in range(4):
        jpos = S0.index(x + 16)
        vt1[:, :, x, :] = Vq[:, jpos].transpose(1, 0, 2)
        vt1[:, :, 4 + x, :] = Vq[:, S0.index(x)].transpose(1, 0, 2)
    vt1 = vt1.reshape(128, NQ * 8 * C)
    scl = np.zeros((128, 4))
    scl[:, 0] = 2.0 * cq[4]
    scl[:, 1] = 2.0 * cq[1]
    scl[:, 2] = 2.0 * cq[0]
    btab = np.ascontiguousarray(
        np.concatenate([vt2, vt1, scl], axis=1)
    ).astype(BFNP)
    return {"xtb": xtb, "btab": btab}


def _postprocess(results, sq, cq, V):
    # loss = 1/12 sum_q sum_i alpha_q[i] * (sum_cls V[i,cls] R_q[i,cls])
    loss = 0.0
    for c in range(NCORES):
        r = np.asarray(results[c]["r_out"], np.float64).reshape(128, NQ, NI, C)
        for s in range(NI):
            gt_ = (4 * c + S0[s]) % 32
            gi = gt_ * 128 + np.arange(128)
            alpha = np.exp(-np.outer(cq, sq[gi]))         # [NQ, 128]
            loss += np.einsum("qp,pc,pqc->", alpha, V[gi], r[:, :, s, :])
    return loss / C


def _run(in_maps, trace=False, **kw):
    global _BUILT
    if _BUILT is None:
        _BUILT = _build_program()
    return run_bass_kernel_spmd(_BUILT, in_maps, list(range(NCORES)), trace=trace, **kw)


def kernel(source, target, source_label, target_logits, _trace=False, _ret_bkr=False):
    X, sq, cq, V, fq, xt8 = _prep(source, target, source_label, target_logits)
    in_maps = [_core_inputs(c, cq, V, fq, xt8) for c in range(NCORES)]
    bkr = None
    for attempt in range(3):
        try:
            bkr = _run(in_maps, trace=_trace)
            break
        except Exception:
            if attempt == 2:
                raise
            import time as _time

            _time.sleep(2.0)
    loss = _postprocess(bkr.results, sq, cq, V)
    out = np.float32(loss)
    if _ret_bkr:
        return out, bkr
    return out


# revision 27
# speedup vs baseline: 1.0546x; 1.0040x over previous
"""LMMD (DSAN local MMD) loss on 8 Trainium2 NeuronCores — cyclic-support V5.

Math (reference):
    X = concat(source, target)                    # [N=4096, D=1024]
    l2[i,j] = max(|x_i|^2 + |x_j|^2 - 2 x_i.x_j, 0)
    bw      = sum(l2) / (N^2 - N) / 4
    K       = sum_q exp(-l2 / (bw * 2^q)),  q = 0..4
    loss    = sum_c v_c^T K v_c / 12,  V = [s_norm; -t_norm]  (rank-12 weights)

V5 design:
  * Cyclic 16-tile support: core c holds X columns for tiles
    (4c + S0) mod 32 with S0 = {0..7, 12..19}.  The 528 unordered
    128-tile pairs partition into 8 identical 68-job lists (60 weight-2
    oriented pairs covering every (difference, residue) cell once, 4
    weight-1 distance-16 jobs computed twice with opposite orientations,
    4 weight-1 diagonals), so every core runs the SAME program on a
    rotated tile set and per-core X DMA halves to 16 KB/partition.
  * Jobs stream through 9 batches (6|8x7|6 jobs).  Per batch: fp8
    DoubleRow gram into one 2-bank PSUM tile, three bias-free ACT exps
    (e4 = exp(2c4 G) full width, e1/e0 heads straight from the gram at
    8x/16x scale), DVE squaring e3/e2/e1-tail, Pool squares the e0 tail
    from DVE's e1 region only (fully decoupled engine chains).  The
    j-side RBF factor exp(-c_q sq_j) is folded into per-q bf16 vt
    tables; the i-side factor is applied on the host.
  * Weighted reduce keeps es stationary (12-wide moving vt), accumulating
    R_q[i, cls] into two PSUM tiles (q>=2 / q<=1) so the high-q drain
    overlaps the low-q matmuls.  PSUM has_written semantics: one
    start per bank, first-touch overwrites via cleared bits.
"""

import numpy as np
import ml_dtypes

import concourse.bass as bass
from concourse import bacc
import concourse.mybir as mybir
import concourse.tile as tile
from concourse.bass_utils import run_bass_kernel_spmd

B = 2048
D = 1024
C = 12
NCORES = 8
N = 2 * B                 # 4096 total samples
NT = N // 128             # 32 j-tiles
NKC = D // 128            # 8 contraction chunks
NKP = NKC // 2            # 4 DoubleRow chunk-pairs
NQ = 5
M = 16                    # tiles in the cyclic support
NI = 8                    # i-side slots (positions 0..7)
WLAG = 3                  # batches of lag between es production and weighted

S0 = list(range(0, 8)) + list(range(12, 20))

# btab layout (bf16): vt2 [5*M*C] | vt1 [5*8*C] | scales [4]
VT2_COLS = NQ * M * C
VT1_COLS = NQ * 8 * C
BT_COLS = VT2_COLS + VT1_COLS + 4

F8NP = ml_dtypes.float8_e4m3
BFNP = ml_dtypes.bfloat16

_BUILT = None


def _plan_jobs():
    """Deterministic job plan: 68 (jpos, ipos, weight, vt1slot) tuples in
    S0-local positions, every global pair covered exactly once."""
    import itertools

    Sset = set(S0)
    pos = {t: i for i, t in enumerate(S0)}
    ILOCAL = set(range(0, 8))
    pairs = []
    for a, b in itertools.combinations(S0, 2):
        if a not in ILOCAL and b not in ILOCAL:
            continue
        d = (b - a) % 32
        cells = set()
        for (base, dd) in ((a, d), (b, (32 - d) % 32)):
            if 1 <= dd <= 15:
                cells.add((dd, base % 4))
        if cells:
            pairs.append(((a, b), sorted(cells)))
    cells_needed = [(d, r) for d in range(1, 16) for r in range(4)]
    cell_idx = {c: i for i, c in enumerate(cells_needed)}
    adj = [[] for _ in cells_needed]
    for pi, (fs, cells) in enumerate(pairs):
        for cc in cells:
            if cc in cell_idx:
                adj[cell_idx[cc]].append(pi)
    for ci in range(len(adj)):
        adj[ci].sort(key=lambda pi: max(pairs[pi][0]))
    match_pair = {}
    match_cell = [None] * len(cells_needed)

    def aug(ci, seen):
        for pi in adj[ci]:
            if pi in seen:
                continue
            seen.add(pi)
            if pi not in match_pair or aug(match_pair[pi], seen):
                match_pair[pi] = ci
                match_cell[ci] = pi
                return True
        return False

    for ci in range(len(cells_needed)):
        assert aug(ci, set())
    jobs = []
    for ci, pi in enumerate(match_cell):
        (a, b) = pairs[pi][0]
        i_t = a if a in ILOCAL else b
        j_t = b if i_t == a else a
        jobs.append((pos[j_t], pos[i_t], 2, -1))
    for x in range(4):                       # d16, weight 1, computed twice
        jobs.append((pos[x + 16], pos[x], 1, x))
    for x in range(4):                       # diagonal, weight 1
        jobs.append((pos[x], pos[x], 1, 4 + x))
    # order by data arrival (4-position DMA chunks), then j for locality
    jobs.sort(key=lambda jb: (max(jb[0] // 4, jb[1] // 4), jb[0], jb[1]))
    return jobs


JOBS = _plan_jobs()
BATCH_SIZES = [6, 8, 8, 8, 8, 8, 8, 8, 4, 2]
assert sum(BATCH_SIZES) == len(JOBS) == 68
NPOOL_FREE = 2            # trailing batches whose e0 tail runs on DVE, not Pool
BATCHES = []
_k = 0
for bs in BATCH_SIZES:
    BATCHES.append(JOBS[_k : _k + bs])
    _k += bs
NB = len(BATCHES)


def _asplit(w):
    # ACT's exp(8sG)/exp(16sG) head widths (e1/e0); Pool squares e0[a:]
    if w <= 256:
        return 64
    return 128 if w <= 768 else 192


def _build_program():
    fp32 = mybir.dt.float32
    bf16 = mybir.dt.bfloat16
    f8 = mybir.dt.float8e4
    Exp = mybir.ActivationFunctionType.Exp
    Copy = mybir.ActivationFunctionType.Copy
    DR = mybir.MatmulPerfMode.DoubleRow

    nc = bacc.Bacc()
    # host-pretransposed: xtb[p, k, t*128+j] = X[(4c+S0[t])*128+j, k*128+p]
    xtb = nc.declare_dram_parameter("xtb", [128, NKC, M * 128], f8, isOutput=False)
    btab = nc.declare_dram_parameter("btab", [128, BT_COLS], bf16, isOutput=False)
    rout = nc.declare_dram_parameter("r_out", [128, NQ * NI * C], fp32, isOutput=True)

    with tile.TileContext(nc) as tc:
        with (
            tc.tile_pool(name="singles", bufs=1) as singles,
            tc.tile_pool(name="epool", bufs=5) as epool,
            tc.tile_pool(name="gpsum", bufs=3, space="PSUM") as gpsum,
            tc.tile_pool(name="rqpsum", bufs=1, space="PSUM") as rqpsum,
        ):
            xtb_sb = singles.tile([128, NKC, M * 128], f8)
            btab_sb = singles.tile([128, BT_COLS], bf16)
            # PE p-state warm-up: ~3us of dummy matmuls on a never-written
            # scratch tile so the first real gram runs at full clock.  The
            # results land in a recycled gpsum generation nobody reads.
            wsrc = singles.tile([128, 2, 512], f8)
            nc.gpsimd.memset(wsrc, 0.0)
            wu = gpsum.tile([128, 1024], fp32, tag="g", name="gwarm")
            for k in range(14):
                nc.tensor.matmul(
                    wu[:, 0:512],
                    lhsT=wsrc[:, :, 0:128],
                    rhs=wsrc,
                    start=(k == 0),
                    stop=(k == 13),
                    perf_mode=DR,
                )
            # DMA stream: first batch's tiles (positions 0-3) in two k-halves
            # so gram m=0,1 starts early; scales early (tiny); remaining tile
            # chunks; the bulk vt table after the second chunk.
            nc.sync.dma_start(out=xtb_sb[:, 0:4, 0:512], in_=xtb[:, 0:4, 0:512])
            nc.sync.dma_start(out=xtb_sb[:, 4:8, 0:512], in_=xtb[:, 4:8, 0:512])
            nc.sync.dma_start(
                out=btab_sb[:, VT2_COLS + VT1_COLS :],
                in_=btab[:, VT2_COLS + VT1_COLS :],
            )
            scl_s = singles.tile([128, 4], fp32)
            nc.vector.tensor_copy(scl_s, btab_sb[:, VT2_COLS + VT1_COLS :])
            warm = singles.tile([128, 4], fp32)
            nc.scalar.activation(warm, scl_s, Exp)
            nc.sync.dma_start(out=xtb_sb[:, :, 512:1024], in_=xtb[:, :, 512:1024])
            nc.sync.dma_start(
                out=btab_sb[:, 0 : VT2_COLS + VT1_COLS],
                in_=btab[:, 0 : VT2_COLS + VT1_COLS],
            )
            nc.sync.dma_start(out=xtb_sb[:, :, 1024:1536], in_=xtb[:, :, 1024:1536])
            nc.sync.dma_start(out=xtb_sb[:, :, 1536:2048], in_=xtb[:, :, 1536:2048])

            # R accumulators: hi = q {4,3,2}, lo = q {1,0}; one bank each
            rq_hi = rqpsum.tile([128, 3 * NI * C], fp32, tag="rqh", name="rq_hi")
            rq_lo = rqpsum.tile([128, 2 * NI * C], fp32, tag="rql", name="rq_lo")

            def rq_slice(q, islot):
                if q >= 2:
                    base = ((q - 2) * NI + islot) * C
                    return rq_hi[:, base : base + C]
                base = (q * NI + islot) * C
                return rq_lo[:, base : base + C]

            first_mm = {"hi": True, "lo": True}
            n_emitted = [0]

            def emit_weighted(bi, es):
                jobs = BATCHES[bi]
                for q in range(NQ - 1, -1, -1):
                    for jj, (jpos, ipos, w, vt1slot) in enumerate(jobs):
                        if w == 2:
                            vb = (q * M + jpos) * C
                        else:
                            vb = VT2_COLS + (q * 8 + vt1slot) * C
                        key = "hi" if q >= 2 else "lo"
                        n_emitted[0] += 1
                        last = n_emitted[0] == NB and False
                        nc.tensor.matmul(
                            rq_slice(q, ipos),
                            lhsT=es[q][:, jj * 128 : (jj + 1) * 128],
                            rhs=btab_sb[:, vb : vb + C],
                            start=first_mm[key],
                            stop=(bi == NB - 1 and q in (2, 0) and jj == len(jobs) - 1),
                        )
                        first_mm[key] = False

            pending = []
            for bi, jobs in enumerate(BATCHES):
                w = len(jobs) * 128
                a = _asplit(w)
                gt = gpsum.tile([128, 1024], fp32, tag="g", name=f"g{bi}")
                for jj, (jpos, ipos, _, _) in enumerate(jobs):
                    for m in range(NKP):
                        nc.tensor.matmul(
                            gt[:, jj * 128 : (jj + 1) * 128],
                            lhsT=xtb_sb[:, 2 * m : 2 * m + 2, jpos * 128 : (jpos + 1) * 128],
                            rhs=xtb_sb[:, 2 * m : 2 * m + 2, ipos * 128 : (ipos + 1) * 128],
                            start=(m == 0),
                            stop=(m == NKP - 1),
                            perf_mode=DR,
                        )
                es = {q: epool.tile([128, 1024], bf16, tag=f"e{q}", name=f"e{q}b{bi}") for q in range(NQ)}
                nc.scalar.activation(
                    es[4][:, 0:w], gt[:, 0:w], Exp,
                    bias=scl_s[:, 3:4], scale=scl_s[:, 0:1],
                )
                nc.scalar.activation(
                    es[1][:, 0:a], gt[:, 0:a], Exp,
                    bias=scl_s[:, 3:4], scale=scl_s[:, 1:2],
                )
                nc.scalar.activation(
                    es[0][:, 0:a], gt[:, 0:a], Exp,
                    bias=scl_s[:, 3:4], scale=scl_s[:, 2:3],
                )
                nc.vector.tensor_mul(es[3][:, 0:w], es[4][:, 0:w], es[4][:, 0:w])
                nc.vector.tensor_mul(es[2][:, 0:w], es[3][:, 0:w], es[3][:, 0:w])
                nc.vector.tensor_mul(es[1][:, a:w], es[2][:, a:w], es[2][:, a:w])
                if bi >= NB - NPOOL_FREE:
                    nc.vector.tensor_mul(es[0][:, a:w], es[1][:, a:w], es[1][:, a:w])
                else:
                    nc.gpsimd.tensor_mul(es[0][:, a:w], es[1][:, a:w], es[1][:, a:w])
                pending.append((bi, es))
                if len(pending) > WLAG:
                    emit_weighted(*pending.pop(0))
            for item in pending:
                emit_weighted(*item)

            # tail: hi drains on ACT and ships while the low-q matmuls still
            # run; lo follows with its own small DMA.
            stg = singles.tile([128, NQ * NI * C], fp32)
            nc.scalar.activation(stg[:, 2 * NI * C :], rq_hi, Copy)
            nc.sync.dma_start(out=rout[:, 2 * NI * C :], in_=stg[:, 2 * NI * C :])
            nc.vector.tensor_copy(stg[:, 0 : 2 * NI * C], rq_lo)
            nc.sync.dma_start(out=rout[:, 0 : 2 * NI * C], in_=stg[:, 0 : 2 * NI * C])

    nc.compile()
    return nc


def _prep(source, target, source_label, target_logits):
    X = np.concatenate([np.asarray(source), np.asarray(target)], axis=0)
    X64 = X.astype(np.float64)
    sq = np.einsum("nd,nd->n", X64, X64)
    colsum = X64.sum(axis=0)
    sum_l2 = 2.0 * N * sq.sum() - 2.0 * (colsum @ colsum)
    bw = sum_l2 / (N * N - N) / (2.0 ** (NQ // 2))
    cq = np.array([1.0 / (bw * 2.0**q) for q in range(NQ)])  # [5]

    sl = np.asarray(source_label, np.float64)
    tl = np.asarray(target_logits, np.float64)
    ssum = sl.sum(0)
    s_norm = np.where(ssum > 0, sl / np.where(ssum > 0, ssum, 1.0), 0.0)
    tsum = tl.sum(0)
    t_norm = np.where(tsum > 0, tl / np.where(tsum > 0, tsum, 1.0), 0.0)
    s_pres = np.zeros(C)
    np.add.at(s_pres, sl.argmax(1), 1.0)
    t_pres = np.zeros(C)
    np.add.at(t_pres, tl.argmax(1), 1.0)
    common = ((s_pres > 0) & (t_pres > 0)).astype(np.float64)
    V = np.concatenate([s_norm * common, -t_norm * common], axis=0)  # [N, C]

    fq = np.exp(-np.outer(cq, sq))                        # [5, N]

    X8 = X.astype(F8NP)                                   # [N, D]
    xt8 = np.ascontiguousarray(
        X8.T.reshape(NKC, 128, N).transpose(1, 0, 2)      # [128, 8, N]
    )
    return X, sq, cq, V, fq, xt8


def _core_inputs(c, cq, V, fq, xt8):
    gtiles = [(4 * c + s) % 32 for s in S0]
    xtb = np.ascontiguousarray(
        xt8.reshape(128, NKC, NT, 128)[:, :, gtiles, :].reshape(128, NKC, M * 128)
    )
    # vt2[q, t] = 2 V f_q at global tile gtiles[t]
    Vt = V.reshape(NT, 128, C)[gtiles]                    # [M, 128, C]
    fqt = fq.reshape(NQ, NT, 128)[:, gtiles]              # [NQ, M, 128]
    Vq = Vt[None] * fqt[..., None]                        # [NQ, M, 128, C]
    vt2 = (2.0 * Vq).transpose(2, 0, 1, 3).reshape(128, NQ * M * C)
    # vt1 slots: 0..3 = d16 jobs (j = position 8+... tile 16+x), 4..7 = diag x
    vt1 = np.zeros((128, NQ, 8, C))
    for x in range(4):
        jpos = S0.index(x + 16)
        vt1[:, :, x, :] = Vq[:, jpos].transpose(1, 0, 2)
        vt1[:, :, 4 + x, :] = Vq[:, S0.index(x)].transpose(1, 0, 2)
    vt1 = vt1.reshape(128, NQ * 8 * C)
    scl = np.zeros((128, 4))
    scl[:, 0] = 2.0 * cq[4]
    scl[:, 1] = 2.0 * cq[1]
    scl[:, 2] = 2.0 * cq[0]
    btab = np.ascontiguousarray(
        np.concatenate([vt2, vt1, scl], axis=1)
    ).astype(BFNP)
    return {"xtb": xtb, "btab": btab}


def _postprocess(results, sq, cq, V):
    # loss = 1/12 sum_q sum_i alpha_q[i] * (sum_cls V[i,cls] R_q[i,cls])
    loss = 0.0
    for c in range(NCORES):
        r = np.asarray(results[c]["r_out"], np.float64).reshape(128, NQ, NI, C)
        for s in range(NI):
            gt_ = (4 * c + S0[s]) % 32
            gi = gt_ * 128 + np.arange(128)
            alpha = np.exp(-np.outer(cq, sq[gi]))         # [NQ, 128]
            loss += np.einsum("qp,pc,pqc->", alpha, V[gi], r[:, :, s, :])
    return loss / C


def _run(in_maps, trace=False, **kw):
    global _BUILT
    if _BUILT is None:
        _BUILT = _build_program()
    return run_bass_kernel_spmd(_BUILT, in_maps, list(range(NCORES)), trace=trace, **kw)


def kernel(source, target, source_label, target_logits, _trace=False, _ret_bkr=False):
    X, sq, cq, V, fq, xt8 = _prep(source, target, source_label, target_logits)
    in_maps = [_core_inputs(c, cq, V, fq, xt8) for c in range(NCORES)]
    bkr = None
    for attempt in range(3):
        try:
            bkr = _run(in_maps, trace=_trace)
            break
        except Exception:
            if attempt == 2:
                raise
            import time as _time

            _time.sleep(2.0)
    loss = _postprocess(bkr.results, sq, cq, V)
    out = np.float32(loss)
    if _ret_bkr:
        return out, bkr
    return out


# revision 32
# speedup vs baseline: 1.0610x; 1.0061x over previous
"""LMMD (DSAN local MMD) loss on 8 Trainium2 NeuronCores — cyclic-support V5.

Math (reference):
    X = concat(source, target)                    # [N=4096, D=1024]
    l2[i,j] = max(|x_i|^2 + |x_j|^2 - 2 x_i.x_j, 0)
    bw      = sum(l2) / (N^2 - N) / 4
    K       = sum_q exp(-l2 / (bw * 2^q)),  q = 0..4
    loss    = sum_c v_c^T K v_c / 12,  V = [s_norm; -t_norm]  (rank-12 weights)

V5 design:
  * Cyclic 16-tile support: core c holds X columns for tiles
    (4c + S0) mod 32 with S0 = {0..7, 12..19}.  The 528 unordered
    128-tile pairs partition into 8 identical 68-job lists (60 weight-2
    oriented pairs covering every (difference, residue) cell once, 4
    weight-1 distance-16 jobs computed twice with opposite orientations,
    4 weight-1 diagonals), so every core runs the SAME program on a
    rotated tile set and per-core X DMA halves to 16 KB/partition.
  * Jobs stream through 9 batches (6|8x7|6 jobs).  Per batch: fp8
    DoubleRow gram into one 2-bank PSUM tile, three bias-free ACT exps
    (e4 = exp(2c4 G) full width, e1/e0 heads straight from the gram at
    8x/16x scale), DVE squaring e3/e2/e1-tail, Pool squares the e0 tail
    from DVE's e1 region only (fully decoupled engine chains).  The
    j-side RBF factor exp(-c_q sq_j) is folded into per-q bf16 vt
    tables; the i-side factor is applied on the host.
  * Weighted reduce keeps es stationary (12-wide moving vt), accumulating
    R_q[i, cls] into two PSUM tiles (q>=2 / q<=1) so the high-q drain
    overlaps the low-q matmuls.  PSUM has_written semantics: one
    start per bank, first-touch overwrites via cleared bits.
"""

import numpy as np
import ml_dtypes

import concourse.bass as bass
from concourse import bacc
import concourse.mybir as mybir
import concourse.tile as tile
from concourse.bass_utils import run_bass_kernel_spmd

B = 2048
D = 1024
C = 12
NCORES = 8
N = 2 * B                 # 4096 total samples
NT = N // 128             # 32 j-tiles
NKC = D // 128            # 8 contraction chunks
NKP = NKC // 2            # 4 DoubleRow chunk-pairs
NQ = 5
M = 16                    # tiles in the cyclic support
NI = 8                    # i-side slots (positions 0..7)
WLAG = 4                  # batches of lag between es production and weighted

S0 = list(range(0, 8)) + list(range(12, 20))

# btab layout (bf16): vt2 [5*M*C] | vt1 [5*8*C] | scales [4]
VT2_COLS = NQ * M * C
VT1_COLS = NQ * 8 * C
BT_COLS = VT2_COLS + VT1_COLS + 4

F8NP = ml_dtypes.float8_e4m3
BFNP = ml_dtypes.bfloat16

_BUILT = None


def _plan_jobs():
    """Deterministic job plan: 68 (jpos, ipos, weight, vt1slot) tuples in
    S0-local positions, every global pair covered exactly once."""
    import itertools

    Sset = set(S0)
    pos = {t: i for i, t in enumerate(S0)}
    ILOCAL = set(range(0, 8))
    pairs = []
    for a, b in itertools.combinations(S0, 2):
        if a not in ILOCAL and b not in ILOCAL:
            continue
        d = (b - a) % 32
        cells = set()
        for (base, dd) in ((a, d), (b, (32 - d) % 32)):
            if 1 <= dd <= 15:
                cells.add((dd, base % 4))
        if cells:
            pairs.append(((a, b), sorted(cells)))
    cells_needed = [(d, r) for d in range(1, 16) for r in range(4)]
    cell_idx = {c: i for i, c in enumerate(cells_needed)}
    adj = [[] for _ in cells_needed]
    for pi, (fs, cells) in enumerate(pairs):
        for cc in cells:
            if cc in cell_idx:
                adj[cell_idx[cc]].append(pi)
    for ci in range(len(adj)):
        adj[ci].sort(key=lambda pi: max(pairs[pi][0]))
    match_pair = {}
    match_cell = [None] * len(cells_needed)

    def aug(ci, seen):
        for pi in adj[ci]:
            if pi in seen:
                continue
            seen.add(pi)
            if pi not in match_pair or aug(match_pair[pi], seen):
                match_pair[pi] = ci
                match_cell[ci] = pi
                return True
        return False

    for ci in range(len(cells_needed)):
        assert aug(ci, set())
    jobs = []
    for ci, pi in enumerate(match_cell):
        (a, b) = pairs[pi][0]
        i_t = a if a in ILOCAL else b
        j_t = b if i_t == a else a
        jobs.append((pos[j_t], pos[i_t], 2, -1))
    for x in range(4):                       # d16, weight 1, computed twice
        jobs.append((pos[x + 16], pos[x], 1, x))
    for x in range(4):                       # diagonal, weight 1
        jobs.append((pos[x], pos[x], 1, 4 + x))
    # order by data arrival (4-position DMA chunks), then j for locality
    jobs.sort(key=lambda jb: (max(jb[0] // 4, jb[1] // 4), jb[0], jb[1]))
    return jobs


JOBS = _plan_jobs()
BATCH_SIZES = [6, 8, 8, 8, 8, 8, 8, 8, 4, 2]
assert sum(BATCH_SIZES) == len(JOBS) == 68
NPOOL_FREE = 3            # trailing batches whose e0 tail runs on DVE, not Pool
BATCHES = []
_k = 0
for bs in BATCH_SIZES:
    BATCHES.append(JOBS[_k : _k + bs])
    _k += bs
NB = len(BATCHES)


def _asplit(w):
    # ACT's exp(8sG)/exp(16sG) head widths (e1/e0); Pool squares e0[a:]
    if w <= 256:
        return 64
    return 128 if w <= 768 else 192


def _build_program():
    fp32 = mybir.dt.float32
    bf16 = mybir.dt.bfloat16
    f8 = mybir.dt.float8e4
    Exp = mybir.ActivationFunctionType.Exp
    Copy = mybir.ActivationFunctionType.Copy
    DR = mybir.MatmulPerfMode.DoubleRow

    nc = bacc.Bacc()
    # host-pretransposed: xtb[p, k, t*128+j] = X[(4c+S0[t])*128+j, k*128+p]
    xtb = nc.declare_dram_parameter("xtb", [128, NKC, M * 128], f8, isOutput=False)
    btab = nc.declare_dram_parameter("btab", [128, BT_COLS], bf16, isOutput=False)
    rout = nc.declare_dram_parameter("r_out", [128, NQ * NI * C], fp32, isOutput=True)

    with tile.TileContext(nc) as tc:
        with (
            tc.tile_pool(name="singles", bufs=1) as singles,
            tc.tile_pool(name="epool", bufs=6) as epool,
            tc.tile_pool(name="gpsum", bufs=3, space="PSUM") as gpsum,
            tc.tile_pool(name="rqpsum", bufs=1, space="PSUM") as rqpsum,
        ):
            xtb_sb = singles.tile([128, NKC, M * 128], f8)
            btab_sb = singles.tile([128, BT_COLS], bf16)
            # PE p-state warm-up: ~3us of dummy matmuls on a never-written
            # scratch tile so the first real gram runs at full clock.  The
            # results land in a recycled gpsum generation nobody reads.
            wsrc = singles.tile([128, 2, 512], f8)
            nc.gpsimd.memset(wsrc, 0.0)
            wu = gpsum.tile([128, 1024], fp32, tag="g", name="gwarm")
            for k in range(14):
                nc.tensor.matmul(
                    wu[:, 0:512],
                    lhsT=wsrc[:, :, 0:128],
                    rhs=wsrc,
                    start=(k == 0),
                    stop=(k == 13),
                    perf_mode=DR,
                )
            # DMA stream: first batch's tiles (positions 0-3) in two k-halves
            # so gram m=0,1 starts early; scales early (tiny); remaining tile
            # chunks; the bulk vt table after the second chunk.
            nc.sync.dma_start(out=xtb_sb[:, 0:4, 0:512], in_=xtb[:, 0:4, 0:512])
            nc.sync.dma_start(out=xtb_sb[:, 4:8, 0:512], in_=xtb[:, 4:8, 0:512])
            nc.sync.dma_start(
                out=btab_sb[:, VT2_COLS + VT1_COLS :],
                in_=btab[:, VT2_COLS + VT1_COLS :],
            )
            scl_s = singles.tile([128, 4], fp32)
            nc.vector.tensor_copy(scl_s, btab_sb[:, VT2_COLS + VT1_COLS :])
            # Exp-table warm-up reads a const tile so it runs during the DMA
            # head instead of waiting for the scale fetch.
            warm_in = singles.tile([128, 4], fp32)
            nc.gpsimd.memset(warm_in, 0.0)
            warm = singles.tile([128, 4], fp32)
            nc.scalar.activation(warm, warm_in, Exp)
            nc.sync.dma_start(out=xtb_sb[:, :, 512:1024], in_=xtb[:, :, 512:1024])
            nc.sync.dma_start(
                out=btab_sb[:, 0 : VT2_COLS + VT1_COLS],
                in_=btab[:, 0 : VT2_COLS + VT1_COLS],
            )
            nc.sync.dma_start(out=xtb_sb[:, :, 1024:1536], in_=xtb[:, :, 1024:1536])
            nc.sync.dma_start(out=xtb_sb[:, :, 1536:2048], in_=xtb[:, :, 1536:2048])

            # R accumulators: hi = q {4,3,2}, lo = q {1,0}; one bank each
            rq_hi = rqpsum.tile([128, 3 * NI * C], fp32, tag="rqh", name="rq_hi")
            rq_lo = rqpsum.tile([128, 2 * NI * C], fp32, tag="rql", name="rq_lo")

            def rq_slice(q, islot):
                if q >= 2:
                    base = ((q - 2) * NI + islot) * C
                    return rq_hi[:, base : base + C]
                base = (q * NI + islot) * C
                return rq_lo[:, base : base + C]

            first_mm = {"hi": True, "lo": True}
            n_emitted = [0]

            def emit_weighted(bi, es):
                jobs = BATCHES[bi]
                for q in range(NQ - 1, -1, -1):
                    for jj, (jpos, ipos, w, vt1slot) in enumerate(jobs):
                        if w == 2:
                            vb = (q * M + jpos) * C
                        else:
                            vb = VT2_COLS + (q * 8 + vt1slot) * C
                        key = "hi" if q >= 2 else "lo"
                        n_emitted[0] += 1
                        last = n_emitted[0] == NB and False
                        nc.tensor.matmul(
                            rq_slice(q, ipos),
                            lhsT=es[q][:, jj * 128 : (jj + 1) * 128],
                            rhs=btab_sb[:, vb : vb + C],
                            start=first_mm[key],
                            stop=(bi == NB - 1 and q in (2, 0) and jj == len(jobs) - 1),
                        )
                        first_mm[key] = False

            pending = []
            for bi, jobs in enumerate(BATCHES):
                w = len(jobs) * 128
                a = _asplit(w)
                gt = gpsum.tile([128, 1024], fp32, tag="g", name=f"g{bi}")
                for jj, (jpos, ipos, _, _) in enumerate(jobs):
                    for m in range(NKP):
                        nc.tensor.matmul(
                            gt[:, jj * 128 : (jj + 1) * 128],
                            lhsT=xtb_sb[:, 2 * m : 2 * m + 2, jpos * 128 : (jpos + 1) * 128],
                            rhs=xtb_sb[:, 2 * m : 2 * m + 2, ipos * 128 : (ipos + 1) * 128],
                            start=(m == 0),
                            stop=(m == NKP - 1),
                            perf_mode=DR,
                        )
                es = {q: epool.tile([128, 1024], bf16, tag=f"e{q}", name=f"e{q}b{bi}") for q in range(NQ)}
                nc.scalar.activation(
                    es[4][:, 0:w], gt[:, 0:w], Exp,
                    bias=scl_s[:, 3:4], scale=scl_s[:, 0:1],
                )
                nc.scalar.activation(
                    es[1][:, 0:a], gt[:, 0:a], Exp,
                    bias=scl_s[:, 3:4], scale=scl_s[:, 1:2],
                )
                nc.scalar.activation(
                    es[0][:, 0:a], gt[:, 0:a], Exp,
                    bias=scl_s[:, 3:4], scale=scl_s[:, 2:3],
                )
                nc.vector.tensor_mul(es[3][:, 0:w], es[4][:, 0:w], es[4][:, 0:w])
                nc.vector.tensor_mul(es[2][:, 0:w], es[3][:, 0:w], es[3][:, 0:w])
                nc.vector.tensor_mul(es[1][:, a:w], es[2][:, a:w], es[2][:, a:w])
                if bi >= NB - NPOOL_FREE:
                    nc.vector.tensor_mul(es[0][:, a:w], es[1][:, a:w], es[1][:, a:w])
                else:
                    nc.gpsimd.tensor_mul(es[0][:, a:w], es[1][:, a:w], es[1][:, a:w])
                pending.append((bi, es))
                if len(pending) > WLAG:
                    emit_weighted(*pending.pop(0))
            for item in pending:
                emit_weighted(*item)

            # tail: hi drains on ACT while the low-q matmuls still run, lo on
            # DVE, then one DMA.
            stg = singles.tile([128, NQ * NI * C], fp32)
            nc.scalar.activation(stg[:, 2 * NI * C :], rq_hi, Copy)
            nc.vector.tensor_copy(stg[:, 0 : 2 * NI * C], rq_lo)
            nc.sync.dma_start(out=rout[:], in_=stg)

    nc.compile()
    return nc


def _prep(source, target, source_label, target_logits):
    X = np.concatenate([np.asarray(source), np.asarray(target)], axis=0)
    X64 = X.astype(np.float64)
    sq = np.einsum("nd,nd->n", X64, X64)
    colsum = X64.sum(axis=0)
    sum_l2 = 2.0 * N * sq.sum() - 2.0 * (colsum @ colsum)
    bw = sum_l2 / (N * N - N) / (2.0 ** (NQ // 2))
    cq = np.array([1.0 / (bw * 2.0**q) for q in range(NQ)])  # [5]

    sl = np.asarray(source_label, np.float64)
    tl = np.asarray(target_logits, np.float64)
    ssum = sl.sum(0)
    s_norm = np.where(ssum > 0, sl / np.where(ssum > 0, ssum, 1.0), 0.0)
    tsum = tl.sum(0)
    t_norm = np.where(tsum > 0, tl / np.where(tsum > 0, tsum, 1.0), 0.0)
    s_pres = np.zeros(C)
    np.add.at(s_pres, sl.argmax(1), 1.0)
    t_pres = np.zeros(C)
    np.add.at(t_pres, tl.argmax(1), 1.0)
    common = ((s_pres > 0) & (t_pres > 0)).astype(np.float64)
    V = np.concatenate([s_norm * common, -t_norm * common], axis=0)  # [N, C]

    fq = np.exp(-np.outer(cq, sq))                        # [5, N]

    X8 = X.astype(F8NP)                                   # [N, D]
    xt8 = np.ascontiguousarray(
        X8.T.reshape(NKC, 128, N).transpose(1, 0, 2)      # [128, 8, N]
    )
    return X, sq, cq, V, fq, xt8


def _core_inputs(c, cq, V, fq, xt8):
    gtiles = [(4 * c + s) % 32 for s in S0]
    xtb = np.ascontiguousarray(
        xt8.reshape(128, NKC, NT, 128)[:, :, gtiles, :].reshape(128, NKC, M * 128)
    )
    # vt2[q, t] = 2 V f_q at global tile gtiles[t]
    Vt = V.reshape(NT, 128, C)[gtiles]                    # [M, 128, C]
    fqt = fq.reshape(NQ, NT, 128)[:, gtiles]              # [NQ, M, 128]
    Vq = Vt[None] * fqt[..., None]                        # [NQ, M, 128, C]
    vt2 = (2.0 * Vq).transpose(2, 0, 1, 3).reshape(128, NQ * M * C)
    # vt1 slots: 0..3 = d16 jobs (j = position 8+... tile 16+x), 4..7 = diag x
    vt1 = np.zeros((128, NQ, 8, C))
    for x in range(4):
        jpos = S0.index(x + 16)
        vt1[:, :, x, :] = Vq[:, jpos].transpose(1, 0, 2)
        vt1[:, :, 4 + x, :] = Vq[:, S0.index(x)].transpose(1, 0, 2)
    vt1 = vt1.reshape(128, NQ * 8 * C)
    scl = np.zeros((128, 4))
    scl[:, 0] = 2.0 * cq[4]
    scl[:, 1] = 2.0 * cq[1]
    scl[:, 2] = 2.0 * cq[0]
    btab = np.ascontiguousarray(
        np.concatenate([vt2, vt1, scl], axis=1)
    ).astype(BFNP)
    return {"xtb": xtb, "btab": btab}


def _postprocess(results, sq, cq, V):
    # loss = 1/12 sum_q sum_i alpha_q[i] * (sum_cls V[i,cls] R_q[i,cls])
    loss = 0.0
    for c in range(NCORES):
        r = np.asarray(results[c]["r_out"], np.float64).reshape(128, NQ, NI, C)
        for s in range(NI):
            gt_ = (4 * c + S0[s]) % 32
            gi = gt_ * 128 + np.arange(128)
            alpha = np.exp(-np.outer(cq, sq[gi]))         # [NQ, 128]
            loss += np.einsum("qp,pc,pqc->", alpha, V[gi], r[:, :, s, :])
    return loss / C


def _run(in_maps, trace=False, **kw):
    global _BUILT
    if _BUILT is None:
        _BUILT = _build_program()
    return run_bass_kernel_spmd(_BUILT, in_maps, list(range(NCORES)), trace=trace, **kw)


def kernel(source, target, source_label, target_logits, _trace=False, _ret_bkr=False):
    X, sq, cq, V, fq, xt8 = _prep(source, target, source_label, target_logits)
    in_maps = [_core_inputs(c, cq, V, fq, xt8) for c in range(NCORES)]
    bkr = None
    for attempt in range(3):
        try:
            bkr = _run(in_maps, trace=_trace)
            break
        except Exception:
            if attempt == 2:
                raise
            import time as _time

            _time.sleep(2.0)
    loss = _postprocess(bkr.results, sq, cq, V)
    out = np.float32(loss)
    if _ret_bkr:
        return out, bkr
    return out


# revision 37
# speedup vs baseline: 1.0688x; 1.0074x over previous
"""LMMD (DSAN local MMD) loss on 8 Trainium2 NeuronCores — cyclic-support V5.

Math (reference):
    X = concat(source, target)                    # [N=4096, D=1024]
    l2[i,j] = max(|x_i|^2 + |x_j|^2 - 2 x_i.x_j, 0)
    bw      = sum(l2) / (N^2 - N) / 4
    K       = sum_q exp(-l2 / (bw * 2^q)),  q = 0..4
    loss    = sum_c v_c^T K v_c / 12,  V = [s_norm; -t_norm]  (rank-12 weights)

V5 design:
  * Cyclic 16-tile support: core c holds X columns for tiles
    (4c + S0) mod 32 with S0 = {0..7, 12..19}.  The 528 unordered
    128-tile pairs partition into 8 identical 68-job lists (60 weight-2
    oriented pairs covering every (difference, residue) cell once, 4
    weight-1 distance-16 jobs computed twice with opposite orientations,
    4 weight-1 diagonals), so every core runs the SAME program on a
    rotated tile set and per-core X DMA halves to 16 KB/partition.
  * Jobs stream through 9 batches (6|8x7|6 jobs).  Per batch: fp8
    DoubleRow gram into one 2-bank PSUM tile, three bias-free ACT exps
    (e4 = exp(2c4 G) full width, e1/e0 heads straight from the gram at
    8x/16x scale), DVE squaring e3/e2/e1-tail, Pool squares the e0 tail
    from DVE's e1 region only (fully decoupled engine chains).  The
    j-side RBF factor exp(-c_q sq_j) is folded into per-q bf16 vt
    tables; the i-side factor is applied on the host.
  * Weighted reduce keeps es stationary (12-wide moving vt), accumulating
    R_q[i, cls] into two PSUM tiles (q>=2 / q<=1) so the high-q drain
    overlaps the low-q matmuls.  PSUM has_written semantics: one
    start per bank, first-touch overwrites via cleared bits.
"""

import numpy as np
import ml_dtypes

import concourse.bass as bass
from concourse import bacc
import concourse.mybir as mybir
import concourse.tile as tile
from concourse.bass_utils import run_bass_kernel_spmd

B = 2048
D = 1024
C = 12
NCORES = 8
N = 2 * B                 # 4096 total samples
NT = N // 128             # 32 j-tiles
NKC = D // 128            # 8 contraction chunks
NKP = NKC // 2            # 4 DoubleRow chunk-pairs
NQ = 5
M = 16                    # tiles in the cyclic support
NI = 8                    # i-side slots (positions 0..7)
WLAG = 4                  # batches of lag between es production and weighted

S0 = list(range(0, 8)) + list(range(12, 20))

# btab layout (bf16): vt2 [5*M*C] | vt1 [5*8*C] | scales [8]
# scales: [2c4, 2c3, 2c2, 2c1, 2c0, 0(bias), 0, 0]
VT2_COLS = NQ * M * C
VT1_COLS = NQ * 8 * C
NSCL = 8
BT_COLS = VT2_COLS + VT1_COLS + NSCL

F8NP = ml_dtypes.float8_e4m3
BFNP = ml_dtypes.bfloat16

_BUILT = None


def _plan_jobs():
    """Deterministic job plan: 68 (jpos, ipos, weight, vt1slot) tuples in
    S0-local positions, every global pair covered exactly once."""
    import itertools

    Sset = set(S0)
    pos = {t: i for i, t in enumerate(S0)}
    ILOCAL = set(range(0, 8))
    pairs = []
    for a, b in itertools.combinations(S0, 2):
        if a not in ILOCAL and b not in ILOCAL:
            continue
        d = (b - a) % 32
        cells = set()
        for (base, dd) in ((a, d), (b, (32 - d) % 32)):
            if 1 <= dd <= 15:
                cells.add((dd, base % 4))
        if cells:
            pairs.append(((a, b), sorted(cells)))
    cells_needed = [(d, r) for d in range(1, 16) for r in range(4)]
    cell_idx = {c: i for i, c in enumerate(cells_needed)}
    adj = [[] for _ in cells_needed]
    for pi, (fs, cells) in enumerate(pairs):
        for cc in cells:
            if cc in cell_idx:
                adj[cell_idx[cc]].append(pi)
    for ci in range(len(adj)):
        adj[ci].sort(key=lambda pi: max(pairs[pi][0]))
    match_pair = {}
    match_cell = [None] * len(cells_needed)

    def aug(ci, seen):
        for pi in adj[ci]:
            if pi in seen:
                continue
            seen.add(pi)
            if pi not in match_pair or aug(match_pair[pi], seen):
                match_pair[pi] = ci
                match_cell[ci] = pi
                return True
        return False

    for ci in range(len(cells_needed)):
        assert aug(ci, set())
    jobs = []
    for ci, pi in enumerate(match_cell):
        (a, b) = pairs[pi][0]
        i_t = a if a in ILOCAL else b
        j_t = b if i_t == a else a
        jobs.append((pos[j_t], pos[i_t], 2, -1))
    for x in range(4):                       # d16, weight 1, computed twice
        jobs.append((pos[x + 16], pos[x], 1, x))
    for x in range(4):                       # diagonal, weight 1
        jobs.append((pos[x], pos[x], 1, 4 + x))
    # order by data arrival (4-position DMA chunks), then j for locality
    jobs.sort(key=lambda jb: (max(jb[0] // 4, jb[1] // 4), jb[0], jb[1]))
    return jobs


JOBS = _plan_jobs()
BATCH_SIZES = [6, 8, 8, 8, 8, 8, 8, 8, 4, 2]
assert sum(BATCH_SIZES) == len(JOBS) == 68
NPOOL_FREE = 3            # trailing batches whose e0 tail runs on DVE, not Pool
BATCHES = []
_k = 0
for bs in BATCH_SIZES:
    BATCHES.append(JOBS[_k : _k + bs])
    _k += bs
NB = len(BATCHES)


def _asplit(w):
    # ACT's exp(8sG)/exp(16sG) head widths (e1/e0); Pool squares e0[a:]
    if w <= 256:
        return 64
    return 128 if w <= 768 else 192


def _build_program():
    fp32 = mybir.dt.float32
    bf16 = mybir.dt.bfloat16
    f8 = mybir.dt.float8e4
    Exp = mybir.ActivationFunctionType.Exp
    Copy = mybir.ActivationFunctionType.Copy
    DR = mybir.MatmulPerfMode.DoubleRow

    nc = bacc.Bacc()
    # host-pretransposed: xtb[p, k, t*128+j] = X[(4c+S0[t])*128+j, k*128+p]
    xtb = nc.declare_dram_parameter("xtb", [128, NKC, M * 128], f8, isOutput=False)
    btab = nc.declare_dram_parameter("btab", [128, BT_COLS], bf16, isOutput=False)
    rout = nc.declare_dram_parameter("r_out", [128, NQ * NI * C], fp32, isOutput=True)

    with tile.TileContext(nc) as tc:
        with (
            tc.tile_pool(name="singles", bufs=1) as singles,
            tc.tile_pool(name="epool", bufs=6) as epool,
            tc.tile_pool(name="gpsum", bufs=3, space="PSUM") as gpsum,
            tc.tile_pool(name="rqpsum", bufs=1, space="PSUM") as rqpsum,
        ):
            xtb_sb = singles.tile([128, NKC, M * 128], f8)
            btab_sb = singles.tile([128, BT_COLS], bf16)
            # PE p-state warm-up: ~3us of dummy matmuls on a never-written
            # scratch tile so the first real gram runs at full clock.  The
            # results land in a recycled gpsum generation nobody reads.
            wsrc = singles.tile([128, 2, 512], f8)
            nc.gpsimd.memset(wsrc, 0.0)
            wu = gpsum.tile([128, 1024], fp32, tag="g", name="gwarm")
            for k in range(14):
                nc.tensor.matmul(
                    wu[:, 0:512],
                    lhsT=wsrc[:, :, 0:128],
                    rhs=wsrc,
                    start=(k == 0),
                    stop=(k == 13),
                    perf_mode=DR,
                )
            # DMA stream: first batch's tiles (positions 0-3) in two k-halves
            # so gram m=0,1 starts early; scales early (tiny); remaining tile
            # chunks; the bulk vt table after the second chunk.
            nc.sync.dma_start(out=xtb_sb[:, 0:4, 0:512], in_=xtb[:, 0:4, 0:512])
            nc.sync.dma_start(out=xtb_sb[:, 4:8, 0:512], in_=xtb[:, 4:8, 0:512])
            nc.sync.dma_start(
                out=btab_sb[:, VT2_COLS + VT1_COLS :],
                in_=btab[:, VT2_COLS + VT1_COLS :],
            )
            scl_s = singles.tile([128, NSCL], fp32)
            nc.vector.tensor_copy(scl_s, btab_sb[:, VT2_COLS + VT1_COLS :])
            # Exp-table warm-up reads a const tile so it runs during the DMA
            # head instead of waiting for the scale fetch.
            warm_in = singles.tile([128, 4], fp32)
            nc.gpsimd.memset(warm_in, 0.0)
            warm = singles.tile([128, 4], fp32)
            nc.scalar.activation(warm, warm_in, Exp)
            nc.sync.dma_start(out=xtb_sb[:, :, 512:1024], in_=xtb[:, :, 512:1024])
            nc.sync.dma_start(
                out=btab_sb[:, 0 : VT2_COLS + VT1_COLS],
                in_=btab[:, 0 : VT2_COLS + VT1_COLS],
            )
            nc.sync.dma_start(out=xtb_sb[:, :, 1024:1536], in_=xtb[:, :, 1024:1536])
            nc.sync.dma_start(out=xtb_sb[:, :, 1536:2048], in_=xtb[:, :, 1536:2048])

            # R accumulators: hi = q {4,3,2}, lo = q {1,0}; one bank each
            rq_hi = rqpsum.tile([128, 3 * NI * C], fp32, tag="rqh", name="rq_hi")
            rq_lo = rqpsum.tile([128, 2 * NI * C], fp32, tag="rql", name="rq_lo")

            def rq_slice(q, islot):
                if q >= 2:
                    base = ((q - 2) * NI + islot) * C
                    return rq_hi[:, base : base + C]
                base = (q * NI + islot) * C
                return rq_lo[:, base : base + C]

            first_mm = {"hi": True, "lo": True}
            n_emitted = [0]

            def emit_weighted(bi, es):
                jobs = BATCHES[bi]
                for q in range(NQ - 1, -1, -1):
                    for jj, (jpos, ipos, w, vt1slot) in enumerate(jobs):
                        if w == 2:
                            vb = (q * M + jpos) * C
                        else:
                            vb = VT2_COLS + (q * 8 + vt1slot) * C
                        key = "hi" if q >= 2 else "lo"
                        n_emitted[0] += 1
                        last = n_emitted[0] == NB and False
                        nc.tensor.matmul(
                            rq_slice(q, ipos),
                            lhsT=es[q][:, jj * 128 : (jj + 1) * 128],
                            rhs=btab_sb[:, vb : vb + C],
                            start=first_mm[key],
                            stop=(bi == NB - 1 and q in (2, 0) and jj == len(jobs) - 1),
                        )
                        first_mm[key] = False

            pending = []
            for bi, jobs in enumerate(BATCHES):
                w = len(jobs) * 128
                a = w if bi == NB - 1 else _asplit(w)
                gt = gpsum.tile([128, 1024], fp32, tag="g", name=f"g{bi}")
                for jj, (jpos, ipos, _, _) in enumerate(jobs):
                    for m in range(NKP):
                        nc.tensor.matmul(
                            gt[:, jj * 128 : (jj + 1) * 128],
                            lhsT=xtb_sb[:, 2 * m : 2 * m + 2, jpos * 128 : (jpos + 1) * 128],
                            rhs=xtb_sb[:, 2 * m : 2 * m + 2, ipos * 128 : (ipos + 1) * 128],
                            start=(m == 0),
                            stop=(m == NKP - 1),
                            perf_mode=DR,
                        )
                es = {q: epool.tile([128, 1024], bf16, tag=f"e{q}", name=f"e{q}b{bi}") for q in range(NQ)}
                zb = scl_s[:, 5:6]
                nc.scalar.activation(
                    es[4][:, 0:w], gt[:, 0:w], Exp, bias=zb, scale=scl_s[:, 0:1],
                )
                nc.scalar.activation(
                    es[1][:, 0:a], gt[:, 0:a], Exp, bias=zb, scale=scl_s[:, 3:4],
                )
                nc.scalar.activation(
                    es[0][:, 0:a], gt[:, 0:a], Exp, bias=zb, scale=scl_s[:, 4:5],
                )
                if bi == NB - 1:
                    # tail batch: dodge the DVE backlog — e1/e0 heads above
                    # are full-width (a == w), e3/e2 as short DVE squares
                    nc.vector.tensor_mul(es[3][:, 0:w], es[4][:, 0:w], es[4][:, 0:w])
                    nc.vector.tensor_mul(es[2][:, 0:w], es[3][:, 0:w], es[3][:, 0:w])
                else:
                    nc.vector.tensor_mul(es[3][:, 0:w], es[4][:, 0:w], es[4][:, 0:w])
                    nc.vector.tensor_mul(es[2][:, 0:w], es[3][:, 0:w], es[3][:, 0:w])
                    nc.vector.tensor_mul(es[1][:, a:w], es[2][:, a:w], es[2][:, a:w])
                    if bi >= NB - NPOOL_FREE:
                        nc.vector.tensor_mul(es[0][:, a:w], es[1][:, a:w], es[1][:, a:w])
                    else:
                        nc.gpsimd.tensor_mul(es[0][:, a:w], es[1][:, a:w], es[1][:, a:w])
                pending.append((bi, es))
                if len(pending) > WLAG:
                    emit_weighted(*pending.pop(0))
            for item in pending:
                emit_weighted(*item)

            # tail: hi drains on ACT while the low-q matmuls still run, lo on
            # DVE, then one DMA.
            stg = singles.tile([128, NQ * NI * C], fp32)
            nc.scalar.activation(stg[:, 2 * NI * C :], rq_hi, Copy)
            nc.vector.tensor_copy(stg[:, 0 : 2 * NI * C], rq_lo)
            nc.sync.dma_start(out=rout[:], in_=stg)

    nc.compile()
    return nc


def _prep(source, target, source_label, target_logits):
    X = np.concatenate([np.asarray(source), np.asarray(target)], axis=0)
    X64 = X.astype(np.float64)
    sq = np.einsum("nd,nd->n", X64, X64)
    colsum = X64.sum(axis=0)
    sum_l2 = 2.0 * N * sq.sum() - 2.0 * (colsum @ colsum)
    bw = sum_l2 / (N * N - N) / (2.0 ** (NQ // 2))
    cq = np.array([1.0 / (bw * 2.0**q) for q in range(NQ)])  # [5]

    sl = np.asarray(source_label, np.float64)
    tl = np.asarray(target_logits, np.float64)
    ssum = sl.sum(0)
    s_norm = np.where(ssum > 0, sl / np.where(ssum > 0, ssum, 1.0), 0.0)
    tsum = tl.sum(0)
    t_norm = np.where(tsum > 0, tl / np.where(tsum > 0, tsum, 1.0), 0.0)
    s_pres = np.zeros(C)
    np.add.at(s_pres, sl.argmax(1), 1.0)
    t_pres = np.zeros(C)
    np.add.at(t_pres, tl.argmax(1), 1.0)
    common = ((s_pres > 0) & (t_pres > 0)).astype(np.float64)
    V = np.concatenate([s_norm * common, -t_norm * common], axis=0)  # [N, C]

    fq = np.exp(-np.outer(cq, sq))                        # [5, N]

    X8 = X.astype(F8NP)                                   # [N, D]
    xt8 = np.ascontiguousarray(
        X8.T.reshape(NKC, 128, N).transpose(1, 0, 2)      # [128, 8, N]
    )
    return X, sq, cq, V, fq, xt8


def _core_inputs(c, cq, V, fq, xt8):
    gtiles = [(4 * c + s) % 32 for s in S0]
    xtb = np.ascontiguousarray(
        xt8.reshape(128, NKC, NT, 128)[:, :, gtiles, :].reshape(128, NKC, M * 128)
    )
    # vt2[q, t] = 2 V f_q at global tile gtiles[t]
    Vt = V.reshape(NT, 128, C)[gtiles]                    # [M, 128, C]
    fqt = fq.reshape(NQ, NT, 128)[:, gtiles]              # [NQ, M, 128]
    Vq = Vt[None] * fqt[..., None]                        # [NQ, M, 128, C]
    vt2 = (2.0 * Vq).transpose(2, 0, 1, 3).reshape(128, NQ * M * C)
    # vt1 slots: 0..3 = d16 jobs (j = position 8+... tile 16+x), 4..7 = diag x
    vt1 = np.zeros((128, NQ, 8, C))
    for x in range(4):
        jpos = S0.index(x + 16)
        vt1[:, :, x, :] = Vq[:, jpos].transpose(1, 0, 2)
        vt1[:, :, 4 + x, :] = Vq[:, S0.index(x)].transpose(1, 0, 2)
    vt1 = vt1.reshape(128, NQ * 8 * C)
    scl = np.zeros((128, 8))
    for k in range(NQ):
        scl[:, k] = 2.0 * cq[4 - k]
    btab = np.ascontiguousarray(
        np.concatenate([vt2, vt1, scl], axis=1)
    ).astype(BFNP)
    return {"xtb": xtb, "btab": btab}


def _postprocess(results, sq, cq, V):
    # loss = 1/12 sum_q sum_i alpha_q[i] * (sum_cls V[i,cls] R_q[i,cls])
    loss = 0.0
    for c in range(NCORES):
        r = np.asarray(results[c]["r_out"], np.float64).reshape(128, NQ, NI, C)
        for s in range(NI):
            gt_ = (4 * c + S0[s]) % 32
            gi = gt_ * 128 + np.arange(128)
            alpha = np.exp(-np.outer(cq, sq[gi]))         # [NQ, 128]
            loss += np.einsum("qp,pc,pqc->", alpha, V[gi], r[:, :, s, :])
    return loss / C


def _run(in_maps, trace=False, **kw):
    global _BUILT
    if _BUILT is None:
        _BUILT = _build_program()
    return run_bass_kernel_spmd(_BUILT, in_maps, list(range(NCORES)), trace=trace, **kw)


def kernel(source, target, source_label, target_logits, _trace=False, _ret_bkr=False):
    X, sq, cq, V, fq, xt8 = _prep(source, target, source_label, target_logits)
    in_maps = [_core_inputs(c, cq, V, fq, xt8) for c in range(NCORES)]
    bkr = None
    for attempt in range(3):
        try:
            bkr = _run(in_maps, trace=_trace)
            break
        except Exception:
            if attempt == 2:
                raise
            import time as _time

            _time.sleep(2.0)
    loss = _postprocess(bkr.results, sq, cq, V)
    out = np.float32(loss)
    if _ret_bkr:
        return out, bkr
    return out
